# revision 1
# baseline (speedup 1.0000x reference)
"""Trainium2 Bass kernel for nn_LogicalGNNLayer (GNN message passing + MLP).

Computation (reference):
    h = term_emb[heads]; t = term_emb[tails]           # gather  [E,B,D]
    agg = segsum(s*(h+pred), tails) + segsum(s*(t+inv), heads)   # [T,B,D]
    agg += EPS*term_emb
    out = relu(agg @ W1 + b1) @ W2 + b2                # [T,B,D]

Strategy:
  - Shard batch B across 8 cores (data parallel, Bc=512 per core); the
    term/edge structure and MLP weights are replicated.
  - The gather/scatter structure depends only on the tiny heads/tails index
    arrays: read them on the host and bake the (dst, src, sign) message list
    into the kernel as a static program.
  - On-chip layout is transposed: d on partitions, (t, b) on the free axis,
    so the MLP matmuls (which contract D) consume the aggregation output
    directly with no on-device transposes.
  - Aggregation: per-term accumulators acc[k] = EPS*term[k] (DVE tensor_scalar,
    4x) then one fp16 tensor_tensor add per message operand (2x mode).
  - MLP: fp16 matmuls on PE (1 cycle/row) with fp32 PSUM accumulation;
    ReLU+bias / bias epilogues on the scalar engine straight out of PSUM.
  - fp16 on-chip halves DMA traffic (the problem is memory-bound); output is
    computed and stored in fp32.
"""

import numpy as np

import concourse.bass as bass
import concourse.tile as tile
from concourse import bacc, mybir
from concourse.bass_utils import run_bass_kernel_spmd

T, B, D, H, E = 16, 4096, 256, 512, 32
EPS = 0.1
N_CORES = 8
BC = B // N_CORES            # 512 batch per core
NB = T * BC                  # 8192 free-axis span (t, b)
DT = D // 128                # 2 d-tiles
HT = H // 128                # 4 h-tiles
NMSG = 2 * E                 # 64 directed messages
PAIR = 1024                  # MLP column chunk (2 PSUM banks)
G = 4                        # messages per streamed emb tile
F16 = mybir.dt.float16
F32 = mybir.dt.float32

_KERNEL_CACHE = {}


def _messages(heads, tails, signs):
    """Directed message list (dst, src, sign, which_emb, e), sorted by dst."""
    msgs = []
    for e in range(E):
        h, t, s = int(heads[e]), int(tails[e]), float(signs[e])
        assert 0 <= h < T and 0 <= t < T
        msgs.append((t, h, s, 0, e))   # msg_to_tail: acc[t] += s*(term[h]+pred[e])
        msgs.append((h, t, s, 1, e))   # msg_to_head: acc[h] += s*(term[t]+inv[e])
    msgs.sort(key=lambda m: m[0])
    return msgs


def _build(msgs_key, repeats=1):
    """Build + compile the per-core SPMD Bass program for a message structure."""
    key = (msgs_key, repeats)
    if key in _KERNEL_CACHE:
        return _KERNEL_CACHE[key]
    msgs = list(msgs_key)
    AF = mybir.ActivationFunctionType
    OP = mybir.AluOpType

    # groups[k] = list of (msg_idx, src, sign) with dst == k (msg_idx sorted)
    groups = [[] for _ in range(T)]
    for m, (dst, src, s, _w, _e) in enumerate(msgs):
        groups[dst].append((m, src, s))

    nc = bacc.Bacc("TRN2", target_bir_lowering=False, debug=False,
                   num_devices=N_CORES)
    termT = nc.declare_dram_parameter("termT", [D, NB], F16, isOutput=False)
    embT = nc.declare_dram_parameter("embT", [D, NMSG * BC], F16, isOutput=False)
    w1d = nc.declare_dram_parameter("w1", [D, H], F16, isOutput=False)
    w2d = nc.declare_dram_parameter("w2", [H, D], F16, isOutput=False)
    b1d = nc.declare_dram_parameter("b1t", [128, HT], F32, isOutput=False)
    b2d = nc.declare_dram_parameter("b2t", [128, DT], F32, isOutput=False)
    outT = nc.declare_dram_parameter("outT", [D, NB], F32, isOutput=True)

    with nc.allow_low_precision(reason="fp16 on-chip aggregation"), \
            tile.TileContext(nc) as tc, \
            tc.tile_pool(name="const", bufs=1) as cpool, \
            tc.tile_pool(name="term", bufs=1) as tpool, \
            tc.tile_pool(name="acc", bufs=1) as apool, \
            tc.tile_pool(name="emb", bufs=4) as epool, \
            tc.tile_pool(name="hid", bufs=8) as hpool, \
            tc.tile_pool(name="out", bufs=4) as opool, \
            tc.tile_pool(name="psum", bufs=2, space="PSUM") as pspool:

        # ---- persistent loads -------------------------------------------
        w1s = []
        w2s = []
        for dt in range(DT):
            w = cpool.tile([128, H], F16, tag=f"w1_{dt}")
            nc.sync.dma_start(w[:], w1d[dt * 128:(dt + 1) * 128, :])
            w1s.append(w)
        for ht in range(HT):
            w = cpool.tile([128, D], F16, tag=f"w2_{ht}")
            nc.sync.dma_start(w[:], w2d[ht * 128:(ht + 1) * 128, :])
            w2s.append(w)
        b1s = cpool.tile([128, HT], F32, tag="b1")
        nc.sync.dma_start(b1s[:], b1d[:])
        b2s = cpool.tile([128, DT], F32, tag="b2")
        nc.sync.dma_start(b2s[:], b2d[:])

        for rep in range(repeats):
            terms = []
            for dt in range(DT):
                tt = tpool.tile([128, NB], F16, tag=f"term_{dt}")
                nc.sync.dma_start(tt[:], termT[dt * 128:(dt + 1) * 128, :])
                terms.append(tt)

            # ---- aggregation -------------------------------------------
            # acc[dt][k] = EPS*term_k + sum_msgs (s*term_src + s*emb_m)
            accs = [[None] * T for _ in range(DT)]
            for k in range(T):
                for dt in range(DT):
                    a = apool.tile([128, BC], F16, tag=f"acc_{dt}_{k}")
                    accs[dt][k] = a
                    nc.vector.tensor_scalar_mul(
                        a[:], terms[dt][:, k * BC:(k + 1) * BC], EPS)
                grp = groups[k]
                for c0 in range(0, len(grp), G):
                    chunk = grp[c0:c0 + G]
                    m0 = chunk[0][0]
                    cnt = len(chunk)
                    for dt in range(DT):
                        et = epool.tile([128, G * BC], F16, tag="emb")
                        nc.sync.dma_start(
                            et[:, :cnt * BC],
                            embT[dt * 128:(dt + 1) * 128,
                                 m0 * BC:(m0 + cnt) * BC])
                        a = accs[dt][k]
                        for i, (m, src, s) in enumerate(chunk):
                            tsl = terms[dt][:, src * BC:(src + 1) * BC]
                            if s == 1.0:
                                nc.vector.tensor_add(a[:], a[:], tsl)
                            elif s == -1.0:
                                nc.vector.tensor_sub(a[:], a[:], tsl)
                            else:
                                nc.vector.scalar_tensor_tensor(
                                    a[:], tsl, s, a[:], OP.mult, OP.add)
                            # emb was pre-scaled by sign on the host
                            nc.vector.tensor_add(
                                a[:], a[:], et[:, i * BC:(i + 1) * BC])

            # ---- MLP: out = relu(agg@W1+b1)@W2 + b2 --------------------
            for p in range(NB // PAIR):
                cp = p * PAIR
                hids = []
                for ht in range(HT):
                    ps = pspool.tile([128, PAIR], F32, tag="ps1")
                    for sub in range(2):
                        k = 2 * p + sub
                        for dt in range(DT):
                            nc.tensor.matmul(
                                ps[:, sub * 512:(sub + 1) * 512],
                                w1s[dt][:, ht * 128:(ht + 1) * 128],
                                accs[dt][k][:],
                                start=(dt == 0), stop=(dt == DT - 1))
                    hid = hpool.tile([128, PAIR], F16, tag="hid")
                    nc.scalar.activation(hid[:], ps[:], AF.Relu,
                                         bias=b1s[:, ht:ht + 1], scale=1.0)
                    hids.append(hid)
                for dt2 in range(DT):
                    ps2 = pspool.tile([128, PAIR], F32, tag="ps2")
                    for sub in range(2):
                        for ht in range(HT):
                            nc.tensor.matmul(
                                ps2[:, sub * 512:(sub + 1) * 512],
                                w2s[ht][:, dt2 * 128:(dt2 + 1) * 128],
                                hids[ht][:, sub * 512:(sub + 1) * 512],
                                start=(ht == 0), stop=(ht == HT - 1))
                    ot = opool.tile([128, PAIR], F32, tag="ot")
                    nc.scalar.activation(ot[:], ps2[:], AF.Identity,
                                         bias=b2s[:, dt2:dt2 + 1], scale=1.0)
                    nc.sync.dma_start(
                        outT[dt2 * 128:(dt2 + 1) * 128, cp:cp + PAIR], ot[:])

    nc.compile()
    _KERNEL_CACHE[key] = nc
    return nc


def _prep_inputs(term_emb, pred_emb, inv_pred_emb, W1, b1, W2, b2, msgs):
    """Shard/transpose/cast host-side into the per-core device layouts."""
    t16 = term_emb.astype(np.float16)
    emb = np.empty((NMSG, B, D), np.float16)
    for m, (_dst, _src, s, which, e) in enumerate(msgs):
        arr = pred_emb if which == 0 else inv_pred_emb
        if s == 1.0:
            emb[m] = arr[e]
        else:
            emb[m] = s * arr[e]
    w1_16 = np.ascontiguousarray(W1.astype(np.float16))
    w2_16 = np.ascontiguousarray(W2.astype(np.float16))
    b1t = np.ascontiguousarray(b1.astype(np.float32).reshape(HT, 128).T)
    b2t = np.ascontiguousarray(b2.astype(np.float32).reshape(DT, 128).T)
    in_maps = []
    for c in range(N_CORES):
        sl = slice(c * BC, (c + 1) * BC)
        termTc = np.ascontiguousarray(
            t16[:, sl, :].transpose(2, 0, 1)).reshape(D, NB)
        embTc = np.ascontiguousarray(
            emb[:, sl, :].transpose(2, 0, 1)).reshape(D, NMSG * BC)
        in_maps.append(dict(termT=termTc, embT=embTc, w1=w1_16, w2=w2_16,
                            b1t=b1t, b2t=b2t))
    return in_maps


def kernel(term_emb, pred_emb, inv_pred_emb, signs, W1, b1, W2, b2,
           heads, tails):
    term_emb = np.asarray(term_emb, dtype=np.float32)
    pred_emb = np.asarray(pred_emb, dtype=np.float32)
    inv_pred_emb = np.asarray(inv_pred_emb, dtype=np.float32)
    signs = np.asarray(signs, dtype=np.float32)
    W1 = np.asarray(W1, dtype=np.float32)
    b1 = np.asarray(b1, dtype=np.float32)
    W2 = np.asarray(W2, dtype=np.float32)
    b2 = np.asarray(b2, dtype=np.float32)
    heads = np.asarray(heads).astype(np.int64)
    tails = np.asarray(tails).astype(np.int64)

    msgs = _messages(heads, tails, signs)
    nc = _build(tuple(msgs))
    in_maps = _prep_inputs(term_emb, pred_emb, inv_pred_emb, W1, b1, W2, b2,
                           msgs)
    res = run_bass_kernel_spmd(nc, in_maps, list(range(N_CORES)))

    out = np.empty((T, B, D), np.float32)
    for c in range(N_CORES):
        o = res.results[c]["outT"].reshape(D, T, BC).transpose(1, 2, 0)
        out[:, c * BC:(c + 1) * BC, :] = o
    return out


# revision 4
# speedup vs baseline: 5714.9132x; 5714.9132x over previous
"""Trainium2 Bass kernel for nn_LogicalGNNLayer (GNN message passing + MLP).

Computation (reference):
    h = term_emb[heads]; t = term_emb[tails]           # gather  [E,B,D]
    agg = segsum(s*(h+pred), tails) + segsum(s*(t+inv), heads)   # [T,B,D]
    agg += EPS*term_emb
    out = relu(agg @ W1 + b1) @ W2 + b2                # [T,B,D]

Strategy:
  - Shard batch B across 8 cores (data parallel, Bc=512 per core); the
    term/edge structure and MLP weights are replicated.
  - The gather/scatter structure depends only on the tiny heads/tails index
    arrays: read them on the host and bake the (dst, src, sign) message list
    into the kernel as a static program.
  - On-chip layout is transposed: d on partitions, (t, b) on the free axis,
    so the MLP matmuls (which contract D) consume the aggregation output
    directly with no on-device transposes.
  - Aggregation: per-term accumulators acc[k] = EPS*term[k] (DVE tensor_scalar,
    4x) then one fp16 tensor_tensor add per message operand (2x mode).
  - MLP: fp16 matmuls on PE (1 cycle/row) with fp32 PSUM accumulation;
    ReLU+bias / bias epilogues on the scalar engine straight out of PSUM.
  - fp16 on-chip halves DMA traffic (the problem is memory-bound); output is
    computed and stored in fp32.
"""

import numpy as np

import concourse.bass as bass
import concourse.tile as tile
from concourse import bacc, mybir
from concourse.bass_utils import run_bass_kernel_spmd

T, B, D, H, E = 16, 4096, 256, 512, 32
EPS = 0.1
N_CORES = 8
BC = B // N_CORES            # 512 batch per core
NB = T * BC                  # 8192 free-axis span (t, b)
DT = D // 128                # 2 d-tiles
HT = H // 128                # 4 h-tiles
NMSG = 2 * E                 # 64 directed messages
PAIR = 1024                  # MLP column chunk (2 PSUM banks)
G = 4                        # messages per streamed emb tile
F16 = mybir.dt.float16
F32 = mybir.dt.float32

_KERNEL_CACHE = {}


def _messages(heads, tails, signs):
    """Directed message list (dst, src, sign, which_emb, e), sorted by dst."""
    msgs = []
    for e in range(E):
        h, t, s = int(heads[e]), int(tails[e]), float(signs[e])
        assert 0 <= h < T and 0 <= t < T
        msgs.append((t, h, s, 0, e))   # msg_to_tail: acc[t] += s*(term[h]+pred[e])
        msgs.append((h, t, s, 1, e))   # msg_to_head: acc[h] += s*(term[t]+inv[e])
    msgs.sort(key=lambda m: m[0])
    return msgs


def _build(msgs_key, repeats=1, loop=0):
    """Build + compile the per-core SPMD Bass program for a message structure.

    repeats: statically unroll the whole body N times (timing).
    loop: wrap the body in an on-device For_i loop of N iterations (timing).
    """
    key = (msgs_key, repeats, loop)
    if key in _KERNEL_CACHE:
        return _KERNEL_CACHE[key]
    msgs = list(msgs_key)
    AF = mybir.ActivationFunctionType
    OP = mybir.AluOpType

    # groups[k] = list of (msg_idx, src, sign) with dst == k (msg_idx sorted)
    groups = [[] for _ in range(T)]
    for m, (dst, src, s, _w, _e) in enumerate(msgs):
        groups[dst].append((m, src, s))

    nc = bacc.Bacc("TRN2", target_bir_lowering=False, debug=False,
                   num_devices=N_CORES)
    termT = nc.declare_dram_parameter("termT", [D, NB], F16, isOutput=False)
    embT = nc.declare_dram_parameter("embT", [D, NMSG * BC], F16, isOutput=False)
    w1d = nc.declare_dram_parameter("w1", [D, H], F16, isOutput=False)
    w2d = nc.declare_dram_parameter("w2", [H, D], F16, isOutput=False)
    b1d = nc.declare_dram_parameter("b1t", [128, HT], F32, isOutput=False)
    b2d = nc.declare_dram_parameter("b2t", [128, DT], F32, isOutput=False)
    outT = nc.declare_dram_parameter("outT", [D, NB], F32, isOutput=True)

    with nc.allow_low_precision(reason="fp16 on-chip aggregation"), \
            tile.TileContext(nc) as tc, \
            tc.tile_pool(name="const", bufs=1) as cpool, \
            tc.tile_pool(name="term", bufs=1) as tpool, \
            tc.tile_pool(name="acc", bufs=1) as apool, \
            tc.tile_pool(name="emb", bufs=4) as epool, \
            tc.tile_pool(name="hid", bufs=8) as hpool, \
            tc.tile_pool(name="out", bufs=4) as opool, \
            tc.tile_pool(name="psum", bufs=2, space="PSUM") as pspool:

        # ---- persistent loads -------------------------------------------
        w1s = []
        w2s = []
        for dt in range(DT):
            w = cpool.tile([128, H], F16, tag=f"w1_{dt}")
            nc.sync.dma_start(w[:], w1d[dt * 128:(dt + 1) * 128, :])
            w1s.append(w)
        for ht in range(HT):
            w = cpool.tile([128, D], F16, tag=f"w2_{ht}")
            nc.sync.dma_start(w[:], w2d[ht * 128:(ht + 1) * 128, :])
            w2s.append(w)
        b1s = cpool.tile([128, HT], F32, tag="b1")
        nc.sync.dma_start(b1s[:], b1d[:])
        b2s = cpool.tile([128, DT], F32, tag="b2")
        nc.sync.dma_start(b2s[:], b2d[:])

        def body():
            terms = []
            for dt in range(DT):
                tt = tpool.tile([128, NB], F16, tag=f"term_{dt}")
                nc.sync.dma_start(tt[:], termT[dt * 128:(dt + 1) * 128, :])
                terms.append(tt)

            # ---- aggregation -------------------------------------------
            # acc[dt][k] = EPS*term_k + sum_msgs (s*term_src + s*emb_m)
            accs = [[None] * T for _ in range(DT)]
            for k in range(T):
                for dt in range(DT):
                    a = apool.tile([128, BC], F16, tag=f"acc_{dt}_{k}")
                    accs[dt][k] = a
                    nc.vector.tensor_scalar_mul(
                        a[:], terms[dt][:, k * BC:(k + 1) * BC], EPS)
                grp = groups[k]
                for c0 in range(0, len(grp), G):
                    chunk = grp[c0:c0 + G]
                    m0 = chunk[0][0]
                    cnt = len(chunk)
                    for dt in range(DT):
                        et = epool.tile([128, G * BC], F16, tag="emb")
                        nc.sync.dma_start(
                            et[:, :cnt * BC],
                            embT[dt * 128:(dt + 1) * 128,
                                 m0 * BC:(m0 + cnt) * BC])
                        a = accs[dt][k]
                        for i, (m, src, s) in enumerate(chunk):
                            tsl = terms[dt][:, src * BC:(src + 1) * BC]
                            if s == 1.0:
                                nc.vector.tensor_add(a[:], a[:], tsl)
                            elif s == -1.0:
                                nc.vector.tensor_sub(a[:], a[:], tsl)
                            else:
                                nc.vector.scalar_tensor_tensor(
                                    a[:], tsl, s, a[:], OP.mult, OP.add)
                            # emb was pre-scaled by sign on the host
                            nc.vector.tensor_add(
                                a[:], a[:], et[:, i * BC:(i + 1) * BC])

            # ---- MLP: out = relu(agg@W1+b1)@W2 + b2 --------------------
            for p in range(NB // PAIR):
                cp = p * PAIR
                hids = []
                for ht in range(HT):
                    ps = pspool.tile([128, PAIR], F32, tag="ps1")
                    for sub in range(2):
                        k = 2 * p + sub
                        for dt in range(DT):
                            nc.tensor.matmul(
                                ps[:, sub * 512:(sub + 1) * 512],
                                w1s[dt][:, ht * 128:(ht + 1) * 128],
                                accs[dt][k][:],
                                start=(dt == 0), stop=(dt == DT - 1))
                    hid = hpool.tile([128, PAIR], F16, tag="hid")
                    nc.scalar.activation(hid[:], ps[:], AF.Relu,
                                         bias=b1s[:, ht:ht + 1], scale=1.0)
                    hids.append(hid)
                for dt2 in range(DT):
                    ps2 = pspool.tile([128, PAIR], F32, tag="ps2")
                    for sub in range(2):
                        for ht in range(HT):
                            nc.tensor.matmul(
                                ps2[:, sub * 512:(sub + 1) * 512],
                                w2s[ht][:, dt2 * 128:(dt2 + 1) * 128],
                                hids[ht][:, sub * 512:(sub + 1) * 512],
                                start=(ht == 0), stop=(ht == HT - 1))
                    ot = opool.tile([128, PAIR], F32, tag="ot")
                    nc.scalar.activation(ot[:], ps2[:], AF.Identity,
                                         bias=b2s[:, dt2:dt2 + 1], scale=1.0)
                    nc.sync.dma_start(
                        outT[dt2 * 128:(dt2 + 1) * 128, cp:cp + PAIR], ot[:])

        if loop:
            ET = mybir.EngineType
            with tc.For_i(0, loop, 1,
                          hint_engines=(ET.PE, ET.DVE, ET.Activation, ET.SP)):
                body()
        else:
            for _rep in range(repeats):
                body()

    nc.compile()
    _KERNEL_CACHE[key] = nc
    return nc


def _prep_inputs(term_emb, pred_emb, inv_pred_emb, W1, b1, W2, b2, msgs):
    """Shard/transpose/cast host-side into the per-core device layouts."""
    t16 = term_emb.astype(np.float16)
    emb = np.empty((NMSG, B, D), np.float16)
    for m, (_dst, _src, s, which, e) in enumerate(msgs):
        arr = pred_emb if which == 0 else inv_pred_emb
        if s == 1.0:
            emb[m] = arr[e]
        else:
            emb[m] = s * arr[e]
    w1_16 = np.ascontiguousarray(W1.astype(np.float16))
    w2_16 = np.ascontiguousarray(W2.astype(np.float16))
    b1t = np.ascontiguousarray(b1.astype(np.float32).reshape(HT, 128).T)
    b2t = np.ascontiguousarray(b2.astype(np.float32).reshape(DT, 128).T)
    in_maps = []
    for c in range(N_CORES):
        sl = slice(c * BC, (c + 1) * BC)
        termTc = np.ascontiguousarray(
            t16[:, sl, :].transpose(2, 0, 1)).reshape(D, NB)
        embTc = np.ascontiguousarray(
            emb[:, sl, :].transpose(2, 0, 1)).reshape(D, NMSG * BC)
        in_maps.append(dict(termT=termTc, embT=embTc, w1=w1_16, w2=w2_16,
                            b1t=b1t, b2t=b2t))
    return in_maps


def kernel(term_emb, pred_emb, inv_pred_emb, signs, W1, b1, W2, b2,
           heads, tails):
    term_emb = np.asarray(term_emb, dtype=np.float32)
    pred_emb = np.asarray(pred_emb, dtype=np.float32)
    inv_pred_emb = np.asarray(inv_pred_emb, dtype=np.float32)
    signs = np.asarray(signs, dtype=np.float32)
    W1 = np.asarray(W1, dtype=np.float32)
    b1 = np.asarray(b1, dtype=np.float32)
    W2 = np.asarray(W2, dtype=np.float32)
    b2 = np.asarray(b2, dtype=np.float32)
    heads = np.asarray(heads).astype(np.int64)
    tails = np.asarray(tails).astype(np.int64)

    msgs = _messages(heads, tails, signs)
    nc = _build(tuple(msgs))
    in_maps = _prep_inputs(term_emb, pred_emb, inv_pred_emb, W1, b1, W2, b2,
                           msgs)
    res = run_bass_kernel_spmd(nc, in_maps, list(range(N_CORES)))

    out = np.empty((T, B, D), np.float32)
    for c in range(N_CORES):
        o = res.results[c]["outT"].reshape(D, T, BC).transpose(1, 2, 0)
        out[:, c * BC:(c + 1) * BC, :] = o
    return out
